# revision 42
# baseline (speedup 1.0000x reference)
"""Affinity-propagate (SPN) Trainium2 Bass kernel, fp16 pipeline.

Computation (per batch element, see reference):
    w = g / conv3x3_ones(|g|)          # gates, [8, H, W], computed once
    d_{k+1} = max_c conv3x3_ones(w_c * d_k)   # 8 iterations

Distribution: pure data parallel, batch element b -> NeuronCore b (8 cores).

Per-core mapping (H=352 rows as 3 overlapping 128-row tiles):
  - inputs staged fp16 AND pre-padded on host ([C,H,1220] with zero pad
    columns): halves HBM traffic, makes every DMA row a single contiguous
    4.9KB descriptor, and removes all on-chip pad memsets/dtype converts.
  - g is DMA'd straight into the w tiles over 3 queues (SP/Act hardware
    DGE + GpSimd software DGE); |g| via DVE tensor_scalar bitwise-AND
    0x7fff (4x mode); w = g * recip(conv|g|) multiplied in place.
  - 3x3 conv = tri-band matmul over H (fp16 stationary) x 3 PSUM-
    accumulated W-shifts, W chunked 3x406; PSUM->SBUF fp16 evacuation by
    ONE ScalarE copy per channel (multi-bank AP).
  - channel max: pairwise fp16 maxes on DVE chasing the evacuations
    (scalar-engine cadence paces the loop at ~1.55us per channel-tile).
  - seam rows between H tiles fixed with 1-row SBUF->SBUF DMAs.
  - last iteration's final max writes the fp32 staging tile directly;
    the 3 output DMAs go out on 3 different queues in parallel.
"""
from contextlib import ExitStack

import numpy as np

import concourse.bacc as bacc
import concourse.mybir as mybir
import concourse.tile as tile
from concourse.bass_utils import run_bass_kernel_spmd

F32 = mybir.dt.float32
F16 = mybir.dt.float16
U16 = mybir.dt.uint16
ALU = mybir.AluOpType

B, C, H, W = 8, 8, 352, 1216
NCHUNK = 3
CW = 406                        # chunk width; 3 chunks of 406 = 1218 >= W
WB = NCHUNK * CW + 2            # 1220: [0]=pad, 1..1216 data, 1217+ pad
N_ITERS = 8
N_CORES = 8

ROW_BASE = [0, 126, 252]       # first global row of each H tile
ROWS = [128, 128, 100]         # partitions used by each H tile


def _build_nc():
    nc = bacc.Bacc("TRN2", target_bir_lowering=False, debug=False,
                   num_devices=N_CORES)
    g = nc.dram_tensor("g", [C, H, WB], F16, kind="ExternalInput").ap()
    d_in = nc.dram_tensor("d", [H, WB], F16, kind="ExternalInput").ap()
    band = nc.dram_tensor("band", [128, 128], F16, kind="ExternalInput").ap()
    out = nc.dram_tensor("out", [H, W], F32, kind="ExternalOutput").ap()

    with tile.TileContext(nc) as tc, ExitStack() as ctx:
        pw = ctx.enter_context(tc.tile_pool(name="w", bufs=1))
        pd = ctx.enter_context(tc.tile_pool(name="d", bufs=1))
        pc = ctx.enter_context(tc.tile_pool(name="const", bufs=1))
        pa = ctx.enter_context(tc.tile_pool(name="a16", bufs=3))
        pr32 = ctx.enter_context(tc.tile_pool(name="r32", bufs=4))
        pp = ctx.enter_context(tc.tile_pool(name="p", bufs=6))
        pprop = ctx.enter_context(tc.tile_pool(name="prop", bufs=12))
        prm = ctx.enter_context(tc.tile_pool(name="rm", bufs=6))
        po = ctx.enter_context(tc.tile_pool(name="o32", bufs=2))
        psum = ctx.enter_context(tc.tile_pool(name="psum", bufs=2,
                                              space="PSUM"))

        A = pc.tile([128, 128], F16, tag="band", name="bandt")
        nc.sync.dma_start(A[:], band[:])

        wt = [pw.tile([128, C, WB], F16, tag=f"w{t}", name=f"w{t}")
              for t in range(3)]
        dt_ = [pd.tile([128, WB], F16, tag=f"d{t}", name=f"d{t}")
               for t in range(3)]

        g_queues = [nc.sync, nc.scalar, nc.gpsimd]

        # ---- phase 0: w = g / conv3x3_ones(|g|), in place over g ----
        a16_bufs = [pa.tile([128, WB], F16, tag="a16", name=f"a16_{i}")
                    for i in range(3)]
        for buf in a16_bufs:
            nc.vector.memset(buf[:, 0:1], 0.0)
            nc.vector.memset(buf[:, W + 1:WB], 0.0)
        a16_ctr = [0]

        def phase0_pair(pair):
            c0 = 2 * pair
            for t in range(3):
                R, rb = ROWS[t], ROW_BASE[t]
                if pair == 0:
                    # prime the pipeline: per-channel loads on both queues
                    for ci in (0, 1):
                        g_queues[ci].dma_start(
                            wt[t][0:R, c0 + ci:c0 + ci + 1, :],
                            g[c0 + ci:c0 + ci + 1, rb:rb + R, :]
                            .rearrange("c p w -> p c w"))
                else:
                    # late pairs' tile-2 loads ride the idle GpSimd queue,
                    # relieving the two HW queues so pair-3 data lands
                    # before its abs/conv chain needs it
                    if pair >= 2 and t == 2:
                        q = g_queues[2]
                    else:
                        q = g_queues[(pair * 3 + t) % 2]
                    q.dma_start(
                        wt[t][0:R, c0:c0 + 2, :],
                        g[c0:c0 + 2, rb:rb + R, :].rearrange("c p w -> p c w"))
                for c in (c0, c0 + 1):
                    a16 = a16_bufs[a16_ctr[0] % 3]
                    a16_ctr[0] += 1
                    nc.scalar.activation(a16[0:R, 1:W + 1],
                                         wt[t][0:R, c, 1:W + 1],
                                         mybir.ActivationFunctionType.Abs)
                    ps = psum.tile([128, NCHUNK, 512], F32, tag="ps",
                                   name="ps")
                    for k in range(NCHUNK):
                        for s in range(3):
                            nc.tensor.matmul(
                                ps[0:R, k, 0:CW], A[0:R, 0:R],
                                a16[0:R, k * CW + s:k * CW + s + CW],
                                start=(s == 0), stop=(s == 2))
                    r32 = pr32.tile([128, NCHUNK, CW], F32, tag="r32",
                                    name="r32")
                    nc.vector.reciprocal_approx_fast(
                        out=r32[0:R, :, :], in_=ps[0:R, :, 0:CW])
                    # w = g * recip(fp32), in place: channels 0-1 on DVE
                    # (1x), the rest on GpSimd, keeping the phase-0 DVE
                    # chain (recip) short so the head stays DMA/PE-bound
                    rflat = r32[0:R, :, :].rearrange(
                        "p a b -> p (a b)")[:, 0:W]
                    eng = nc.vector if c < 2 else nc.gpsimd
                    eng.tensor_tensor(wt[t][0:R, c, 1:W + 1],
                                      wt[t][0:R, c, 1:W + 1],
                                      rflat, ALU.mult)
            # w seam rows for this channel pair
            c1 = c0 + 2
            nc.sync.dma_start(wt[0][127:128, c0:c1, 1:W + 1],
                              wt[1][1:2, c0:c1, 1:W + 1])
            nc.sync.dma_start(wt[1][0:1, c0:c1, 1:W + 1],
                              wt[0][126:127, c0:c1, 1:W + 1])
            nc.sync.dma_start(wt[1][127:128, c0:c1, 1:W + 1],
                              wt[2][1:2, c0:c1, 1:W + 1])
            nc.sync.dma_start(wt[2][0:1, c0:c1, 1:W + 1],
                              wt[1][126:127, c0:c1, 1:W + 1])

        # ---- one (tile, channel-pair) unit of one iteration ----
        rm_state = {}
        outspec = {0: (0, 127), 1: (1, 127), 2: (1, 100)}

        def iter_tile_pair(t, pair, last=False):
            R = ROWS[t]
            c0 = 2 * pair
            p16 = pp.tile([128, 2, WB], F16, tag="p", name="p16")
            dbc2 = dt_[t][0:R, :].unsqueeze(1).broadcast_to([R, 2, WB])
            nc.vector.tensor_mul(p16[0:R, :, :],
                                 wt[t][0:R, c0:c0 + 2, :], dbc2)
            props = []
            for ci in (0, 1):
                ps = psum.tile([128, NCHUNK, 512], F32, tag="ps", name="ps")
                for kk in range(NCHUNK):
                    for s in range(3):
                        nc.tensor.matmul(
                            ps[0:R, kk, 0:CW], A[0:R, 0:R],
                            p16[0:R, ci, kk * CW + s:kk * CW + s + CW],
                            start=(s == 0), stop=(s == 2))
                prop = pprop.tile([128, NCHUNK * CW], F16, tag="prop",
                                  name="prop")
                nc.scalar.copy(
                    prop[0:R, :].rearrange("p (a b) -> p a b", a=NCHUNK),
                    ps[0:R, :, 0:CW])
                props.append(prop)
            if pair == 0:
                rm = prm.tile([128, W], F16, tag="rm", name="rm")
                nc.vector.tensor_max(rm[0:R, :], props[0][0:R, 0:W],
                                     props[1][0:R, 0:W])
                rm_state[t] = rm
            else:
                pm = prm.tile([128, W], F16, tag="rm", name="pm")
                nc.vector.tensor_max(pm[0:R, :], props[0][0:R, 0:W],
                                     props[1][0:R, 0:W])
                if pair < C // 2 - 1:
                    nc.vector.tensor_max(rm_state[t][0:R, :],
                                         rm_state[t][0:R, :], pm[0:R, :])
                elif not last:
                    nc.vector.tensor_max(dt_[t][0:R, 1:W + 1],
                                         rm_state[t][0:R, :], pm[0:R, :])
                else:
                    r0, r1 = outspec[t]
                    o32 = po.tile([128, W], F32, tag="o32", name="o32")
                    nc.vector.tensor_max(o32[0:R, :],
                                         rm_state[t][0:R, :], pm[0:R, :])
                    gb = ROW_BASE[t] + r0
                    g_queues[t].dma_start(out[gb:gb + (r1 - r0), :],
                                          o32[r0:r1, :])

        def d_seams():
            nc.sync.dma_start(dt_[0][127:128, 1:W + 1], dt_[1][1:2, 1:W + 1])
            nc.sync.dma_start(dt_[1][0:1, 1:W + 1], dt_[0][126:127, 1:W + 1])
            nc.sync.dma_start(dt_[1][127:128, 1:W + 1], dt_[2][1:2, 1:W + 1])
            nc.sync.dma_start(dt_[2][0:1, 1:W + 1], dt_[1][126:127, 1:W + 1])

        # ---- emission schedule ----
        # phase 0 pairs 0..3 with iteration-1 pair-groups woven in two
        # pairs behind, filling PE bubbles of phase 0.
        phase0_pair(0)
        # depth loads on the (otherwise idle) GpSimd software queue; not
        # needed until iteration 1, so keep them off the g-load queues
        for t in range(3):
            R, rb = ROWS[t], ROW_BASE[t]
            nc.gpsimd.dma_start(dt_[t][0:R, :], d_in[rb:rb + R, :])
        phase0_pair(1)
        for t in range(3):
            iter_tile_pair(t, 0)
        phase0_pair(2)
        for t in range(3):
            iter_tile_pair(t, 1)
        phase0_pair(3)
        for pair in (2, 3):
            for t in range(3):
                iter_tile_pair(t, pair)
        d_seams()

        # iterations 2..8, tile-major
        for k in range(1, N_ITERS):
            last = k == N_ITERS - 1
            for t in range(3):
                for pair in range(C // 2):
                    iter_tile_pair(t, pair, last)
            if not last:
                d_seams()

    nc.compile()
    return nc


def _band_matrix():
    a = np.zeros((128, 128), dtype=np.float16)
    idx = np.arange(128)
    a[idx, idx] = 1.0
    a[idx[:-1], idx[:-1] + 1] = 1.0
    a[idx[1:], idx[1:] - 1] = 1.0
    return a


_NC_CACHE = None


def kernel(guidance: np.ndarray, blur_depth: np.ndarray) -> np.ndarray:
    """Full inputs in, full output out. Shards batch across 8 NeuronCores."""
    global _NC_CACHE
    guidance = np.asarray(guidance)
    blur_depth = np.asarray(blur_depth)
    assert guidance.shape == (B, C, H, W), guidance.shape
    assert blur_depth.shape == (B, 1, H, W), blur_depth.shape
    if _NC_CACHE is None:
        _NC_CACHE = _build_nc()
    nc = _NC_CACHE
    band = _band_matrix()
    g16 = np.zeros((B, C, H, WB), dtype=np.float16)
    g16[:, :, :, 1:W + 1] = guidance.astype(np.float16)
    d16 = np.zeros((B, H, WB), dtype=np.float16)
    d16[:, :, 1:W + 1] = blur_depth[:, 0].astype(np.float16)
    in_maps = [
        {"g": g16[b], "d": d16[b], "band": band}
        for b in range(B)
    ]
    res = run_bass_kernel_spmd(nc, in_maps, core_ids=list(range(N_CORES)))
    out = np.stack([res.results[b]["out"] for b in range(B)])[:, None]
    return out.astype(np.float32)


# revision 43
# speedup vs baseline: 1.0144x; 1.0144x over previous
"""Affinity-propagate (SPN) Trainium2 Bass kernel, fp16 pipeline.

Computation (per batch element, see reference):
    w = g / conv3x3_ones(|g|)          # gates, [8, H, W], computed once
    d_{k+1} = max_c conv3x3_ones(w_c * d_k)   # 8 iterations

Distribution: pure data parallel, batch element b -> NeuronCore b (8 cores).

Per-core mapping (H=352 rows as 3 overlapping 128-row tiles):
  - inputs staged fp16 AND pre-padded on host ([C,H,1220] with zero pad
    columns): halves HBM traffic, makes every DMA row a single contiguous
    4.9KB descriptor, and removes all on-chip pad memsets/dtype converts.
  - g is DMA'd straight into the w tiles over 3 queues (SP/Act hardware
    DGE + GpSimd software DGE); |g| via DVE tensor_scalar bitwise-AND
    0x7fff (4x mode); w = g * recip(conv|g|) multiplied in place.
  - 3x3 conv = tri-band matmul over H (fp16 stationary) x 3 PSUM-
    accumulated W-shifts, W chunked 3x406; PSUM->SBUF fp16 evacuation by
    ONE ScalarE copy per channel (multi-bank AP).
  - channel max: pairwise fp16 maxes on DVE chasing the evacuations
    (scalar-engine cadence paces the loop at ~1.55us per channel-tile).
  - seam rows between H tiles fixed with 1-row SBUF->SBUF DMAs.
  - last iteration's final max writes the fp32 staging tile directly;
    the 3 output DMAs go out on 3 different queues in parallel.
"""
from contextlib import ExitStack

import numpy as np

import concourse.bacc as bacc
import concourse.mybir as mybir
import concourse.tile as tile
from concourse.bass_utils import run_bass_kernel_spmd

F32 = mybir.dt.float32
F16 = mybir.dt.float16
U16 = mybir.dt.uint16
ALU = mybir.AluOpType

B, C, H, W = 8, 8, 352, 1216
NCHUNK = 3
CW = 406                        # chunk width; 3 chunks of 406 = 1218 >= W
WB = NCHUNK * CW + 2            # 1220: [0]=pad, 1..1216 data, 1217+ pad
N_ITERS = 8
N_CORES = 8

ROW_BASE = [0, 126, 252]       # first global row of each H tile
ROWS = [128, 128, 100]         # partitions used by each H tile


def _build_nc():
    nc = bacc.Bacc("TRN2", target_bir_lowering=False, debug=False,
                   num_devices=N_CORES)
    g = nc.dram_tensor("g", [C, H, WB], F16, kind="ExternalInput").ap()
    d_in = nc.dram_tensor("d", [H, WB], F16, kind="ExternalInput").ap()
    band = nc.dram_tensor("band", [128, 128], F16, kind="ExternalInput").ap()
    out = nc.dram_tensor("out", [H, W], F32, kind="ExternalOutput").ap()

    with tile.TileContext(nc) as tc, ExitStack() as ctx:
        pw = ctx.enter_context(tc.tile_pool(name="w", bufs=1))
        pd = ctx.enter_context(tc.tile_pool(name="d", bufs=1))
        pc = ctx.enter_context(tc.tile_pool(name="const", bufs=1))
        pa = ctx.enter_context(tc.tile_pool(name="a16", bufs=3))
        pr32 = ctx.enter_context(tc.tile_pool(name="r32", bufs=4))
        pp = ctx.enter_context(tc.tile_pool(name="p", bufs=6))
        pprop = ctx.enter_context(tc.tile_pool(name="prop", bufs=12))
        prm = ctx.enter_context(tc.tile_pool(name="rm", bufs=6))
        po = ctx.enter_context(tc.tile_pool(name="o32", bufs=2))
        psum = ctx.enter_context(tc.tile_pool(name="psum", bufs=2,
                                              space="PSUM"))

        A = pc.tile([128, 128], F16, tag="band", name="bandt")
        nc.sync.dma_start(A[:], band[:])

        wt = [pw.tile([128, C, WB], F16, tag=f"w{t}", name=f"w{t}")
              for t in range(3)]
        dt_ = [pd.tile([128, WB], F16, tag=f"d{t}", name=f"d{t}")
               for t in range(3)]

        g_queues = [nc.sync, nc.scalar, nc.gpsimd]

        # ---- phase 0: w = g / conv3x3_ones(|g|), in place over g ----
        a16_bufs = [pa.tile([128, WB], F16, tag="a16", name=f"a16_{i}")
                    for i in range(3)]
        for buf in a16_bufs:
            nc.vector.memset(buf[:, 0:1], 0.0)
            nc.vector.memset(buf[:, W + 1:WB], 0.0)
        a16_ctr = [0]

        def phase0_pair(pair):
            c0 = 2 * pair
            for t in range(3):
                R, rb = ROWS[t], ROW_BASE[t]
                if pair == 0:
                    # prime the pipeline: per-channel loads on both queues
                    for ci in (0, 1):
                        g_queues[ci].dma_start(
                            wt[t][0:R, c0 + ci:c0 + ci + 1, :],
                            g[c0 + ci:c0 + ci + 1, rb:rb + R, :]
                            .rearrange("c p w -> p c w"))
                else:
                    q = g_queues[(pair * 3 + t) % 2]
                    q.dma_start(
                        wt[t][0:R, c0:c0 + 2, :],
                        g[c0:c0 + 2, rb:rb + R, :].rearrange("c p w -> p c w"))
                for c in (c0, c0 + 1):
                    a16 = a16_bufs[a16_ctr[0] % 3]
                    a16_ctr[0] += 1
                    nc.scalar.activation(a16[0:R, 1:W + 1],
                                         wt[t][0:R, c, 1:W + 1],
                                         mybir.ActivationFunctionType.Abs)
                    ps = psum.tile([128, NCHUNK, 512], F32, tag="ps",
                                   name="ps")
                    for k in range(NCHUNK):
                        for s in range(3):
                            nc.tensor.matmul(
                                ps[0:R, k, 0:CW], A[0:R, 0:R],
                                a16[0:R, k * CW + s:k * CW + s + CW],
                                start=(s == 0), stop=(s == 2))
                    r32 = pr32.tile([128, NCHUNK, CW], F32, tag="r32",
                                    name="r32")
                    nc.vector.reciprocal_approx_fast(
                        out=r32[0:R, :, :], in_=ps[0:R, :, 0:CW])
                    # w = g * recip(fp32), in place: channels 0-1 on DVE
                    # (1x), the rest on GpSimd, keeping the phase-0 DVE
                    # chain (recip) short so the head stays DMA/PE-bound
                    rflat = r32[0:R, :, :].rearrange(
                        "p a b -> p (a b)")[:, 0:W]
                    eng = nc.vector if c < 2 else nc.gpsimd
                    eng.tensor_tensor(wt[t][0:R, c, 1:W + 1],
                                      wt[t][0:R, c, 1:W + 1],
                                      rflat, ALU.mult)
            # w seam rows for this channel pair
            c1 = c0 + 2
            nc.sync.dma_start(wt[0][127:128, c0:c1, 1:W + 1],
                              wt[1][1:2, c0:c1, 1:W + 1])
            nc.sync.dma_start(wt[1][0:1, c0:c1, 1:W + 1],
                              wt[0][126:127, c0:c1, 1:W + 1])
            nc.sync.dma_start(wt[1][127:128, c0:c1, 1:W + 1],
                              wt[2][1:2, c0:c1, 1:W + 1])
            nc.sync.dma_start(wt[2][0:1, c0:c1, 1:W + 1],
                              wt[1][126:127, c0:c1, 1:W + 1])

        # ---- one (tile, channel-pair) unit of one iteration ----
        rm_state = {}
        outspec = {0: (0, 127), 1: (1, 127), 2: (1, 100)}

        def iter_tile_pair(t, pair, last=False):
            R = ROWS[t]
            c0 = 2 * pair
            p16 = pp.tile([128, 2, WB], F16, tag="p", name="p16")
            dbc2 = dt_[t][0:R, :].unsqueeze(1).broadcast_to([R, 2, WB])
            nc.vector.tensor_mul(p16[0:R, :, :],
                                 wt[t][0:R, c0:c0 + 2, :], dbc2)
            props = []
            for ci in (0, 1):
                ps = psum.tile([128, NCHUNK, 512], F32, tag="ps", name="ps")
                for kk in range(NCHUNK):
                    for s in range(3):
                        nc.tensor.matmul(
                            ps[0:R, kk, 0:CW], A[0:R, 0:R],
                            p16[0:R, ci, kk * CW + s:kk * CW + s + CW],
                            start=(s == 0), stop=(s == 2))
                prop = pprop.tile([128, NCHUNK * CW], F16, tag="prop",
                                  name="prop")
                nc.scalar.copy(
                    prop[0:R, :].rearrange("p (a b) -> p a b", a=NCHUNK),
                    ps[0:R, :, 0:CW])
                props.append(prop)
            if pair == 0:
                rm = prm.tile([128, W], F16, tag="rm", name="rm")
                nc.vector.tensor_max(rm[0:R, :], props[0][0:R, 0:W],
                                     props[1][0:R, 0:W])
                rm_state[t] = rm
            else:
                pm = prm.tile([128, W], F16, tag="rm", name="pm")
                nc.vector.tensor_max(pm[0:R, :], props[0][0:R, 0:W],
                                     props[1][0:R, 0:W])
                if pair < C // 2 - 1:
                    nc.vector.tensor_max(rm_state[t][0:R, :],
                                         rm_state[t][0:R, :], pm[0:R, :])
                elif not last:
                    nc.vector.tensor_max(dt_[t][0:R, 1:W + 1],
                                         rm_state[t][0:R, :], pm[0:R, :])
                else:
                    r0, r1 = outspec[t]
                    o32 = po.tile([128, W], F32, tag="o32", name="o32")
                    nc.vector.tensor_max(o32[0:R, :],
                                         rm_state[t][0:R, :], pm[0:R, :])
                    gb = ROW_BASE[t] + r0
                    g_queues[t].dma_start(out[gb:gb + (r1 - r0), :],
                                          o32[r0:r1, :])

        def d_seams():
            nc.sync.dma_start(dt_[0][127:128, 1:W + 1], dt_[1][1:2, 1:W + 1])
            nc.sync.dma_start(dt_[1][0:1, 1:W + 1], dt_[0][126:127, 1:W + 1])
            nc.sync.dma_start(dt_[1][127:128, 1:W + 1], dt_[2][1:2, 1:W + 1])
            nc.sync.dma_start(dt_[2][0:1, 1:W + 1], dt_[1][126:127, 1:W + 1])

        # ---- emission schedule ----
        # phase 0 pairs 0..3 with iteration-1 pair-groups woven in two
        # pairs behind, filling PE bubbles of phase 0.
        phase0_pair(0)
        # depth loads on the (otherwise idle) GpSimd software queue; not
        # needed until iteration 1, so keep them off the g-load queues
        for t in range(3):
            R, rb = ROWS[t], ROW_BASE[t]
            nc.gpsimd.dma_start(dt_[t][0:R, :], d_in[rb:rb + R, :])
        phase0_pair(1)
        for t in range(3):
            iter_tile_pair(t, 0)
        phase0_pair(2)
        for t in range(3):
            iter_tile_pair(t, 1)
        phase0_pair(3)
        for pair in (2, 3):
            for t in range(3):
                iter_tile_pair(t, pair)
        d_seams()

        # iterations 2..8, tile-major
        for k in range(1, N_ITERS):
            last = k == N_ITERS - 1
            for t in range(3):
                for pair in range(C // 2):
                    iter_tile_pair(t, pair, last)
            if not last:
                d_seams()

    nc.compile()
    return nc


def _band_matrix():
    a = np.zeros((128, 128), dtype=np.float16)
    idx = np.arange(128)
    a[idx, idx] = 1.0
    a[idx[:-1], idx[:-1] + 1] = 1.0
    a[idx[1:], idx[1:] - 1] = 1.0
    return a


_NC_CACHE = None


def kernel(guidance: np.ndarray, blur_depth: np.ndarray) -> np.ndarray:
    """Full inputs in, full output out. Shards batch across 8 NeuronCores."""
    global _NC_CACHE
    guidance = np.asarray(guidance)
    blur_depth = np.asarray(blur_depth)
    assert guidance.shape == (B, C, H, W), guidance.shape
    assert blur_depth.shape == (B, 1, H, W), blur_depth.shape
    if _NC_CACHE is None:
        _NC_CACHE = _build_nc()
    nc = _NC_CACHE
    band = _band_matrix()
    g16 = np.zeros((B, C, H, WB), dtype=np.float16)
    g16[:, :, :, 1:W + 1] = guidance.astype(np.float16)
    d16 = np.zeros((B, H, WB), dtype=np.float16)
    d16[:, :, 1:W + 1] = blur_depth[:, 0].astype(np.float16)
    in_maps = [
        {"g": g16[b], "d": d16[b], "band": band}
        for b in range(B)
    ]
    res = run_bass_kernel_spmd(nc, in_maps, core_ids=list(range(N_CORES)))
    out = np.stack([res.results[b]["out"] for b in range(B)])[:, None]
    return out.astype(np.float32)
